# revision 1
# baseline (speedup 1.0000x reference)
"""HALE attention (local windowed SDPA + chunked causal linear attention with
multiscale Haar context + adaptive gate) on 8 Trainium2 NeuronCores.

Sharding (B=1, so no batch DP):
  - 16 heads -> 2 heads per core (tensor-parallel over heads), packed into the
    128-partition dim for the q/k/v/local projections, the chunked
    linear-attention recurrence, and the 4 Haar-level recurrences.
  - Tail (gate + out_proj) is sequence-parallel: one AllToAll redistributes the
    per-head outputs (diff=local-glob, glob) from head-sharded to
    sequence-sharded; each core then computes gate/alpha/mixed/out_proj for its
    256 rows against host-pre-transposed Wg/Wo. Output rows concatenated on
    the host.

Layout notes (contraction dim must sit on partitions for both matmul operands):
  - x^T built on-device via PE transposes; projections emit q^T/k^T/... as
    [128 = 2 heads x 64, 2048].
  - Linear attention per chunk (C=128): A^T = kp^T.T @ qp^T, masked on DVE;
    O_aug = A^T_m.T @ v_aug + qp^T.T @ S_aug accumulated in one PSUM tile.
    v_aug/S_aug carry an all-ones 65th column so the normalizer appears in
    O_aug[:, 64] for free. State update S_aug += k_nat.T @ v_aug.
  - Haar block means are matmuls against constant per-level prefix-mean
    matrices M_l; the per-level Dh x Dh projections are block-diagonal
    (2 heads) constant matmuls emitted in both ^T and natural orientations.
"""

import numpy as np
from contextlib import ExitStack

import concourse.bass as bass
import concourse.bacc as bacc
import concourse.tile as tile
import concourse.mybir as mybir
from concourse.bass_utils import run_bass_kernel_spmd

F32 = mybir.dt.float32
AF = mybir.ActivationFunctionType
OP = mybir.AluOpType

NCORES = 8
N = 2048
DM = 1024
H = 16
DH = 64
L = 4
CH = 128
NCH = N // CH
WIN = 64
NSL = N // NCORES
EPS = 1e-6

_CACHE = {}


def _host_constants():
    ident = np.eye(128, dtype=np.float32)
    ck = np.arange(CH)[:, None]
    cq = np.arange(CH)[None, :]
    maskT = (ck <= cq).astype(np.float32)
    prev = (ck >= cq + WIN + 1).astype(np.float32)
    cur = ((ck <= cq) & (ck >= cq - (WIN - 1))).astype(np.float32)
    lmask = np.concatenate([prev, cur], axis=1)
    Ml = np.zeros((L, CH, CH), dtype=np.float32)
    for lv in range(L):
        b = 2 ** (lv + 1)
        m = np.arange(CH)[:, None]
        n = np.arange(CH)[None, :]
        Ml[lv] = np.where(((m // b) == (n // b)) & (m <= n),
                          1.0 / (n % b + 1.0), 0.0)
    return ident, maskT, lmask, Ml


def _blockdiag2(a):
    z = np.zeros((128, 128), dtype=np.float32)
    z[:64, :64] = a
    z[64:, 64:] = a
    return z


def _build_nc():
    nc = bacc.Bacc("TRN2", target_bir_lowering=False, debug=False,
                   num_devices=NCORES)

    x_d = nc.dram_tensor("x", [N, DM], F32, kind="ExternalInput")
    wT = {p: nc.dram_tensor(f"w{p}T", [DM, 128], F32, kind="ExternalInput")
          for p in ("q", "k", "v", "kl", "vl")}
    bdWkT_d = nc.dram_tensor("bdWkT", [L, 128, 128], F32, kind="ExternalInput")
    bdWvT_d = nc.dram_tensor("bdWvT", [L, 128, 128], F32, kind="ExternalInput")
    Ml_d = nc.dram_tensor("Ml", [L, 128, 128], F32, kind="ExternalInput")
    maskT_d = nc.dram_tensor("maskT", [128, 128], F32, kind="ExternalInput")
    lmask_d = nc.dram_tensor("lmask", [128, 256], F32, kind="ExternalInput")
    ident_d = nc.dram_tensor("ident", [128, 128], F32, kind="ExternalInput")
    wgT_d = nc.dram_tensor("wgT", [2 * DM, DM], F32, kind="ExternalInput")
    woT_d = nc.dram_tensor("woT", [DM, DM], F32, kind="ExternalInput")
    wgo_d = nc.dram_tensor("wgo", [DM, 1], F32, kind="ExternalInput")
    bg_d = nc.dram_tensor("bg", [1, DM], F32, kind="ExternalInput")
    bo_d = nc.dram_tensor("bo", [1, DM], F32, kind="ExternalInput")
    bgo_d = nc.dram_tensor("bgo", [1, 1], F32, kind="ExternalInput")
    hs_d = nc.dram_tensor("hscale", [1, L], F32, kind="ExternalInput")
    out_d = nc.dram_tensor("out", [NSL, DM], F32, kind="ExternalOutput")

    # [dest, tensor(diff,glob), chunk, 128, 128]
    a2a_in = nc.dram_tensor("a2a_in", [NCORES, 2, 2, 128, 128], F32)
    a2a_out = nc.dram_tensor("a2a_out", [NCORES, 2, 2, 128, 128], F32)

    with tile.TileContext(nc) as tc, ExitStack() as root:
        cpool = root.enter_context(tc.tile_pool(name="consts", bufs=1))
        persist = root.enter_context(tc.tile_pool(name="persist", bufs=1))

        ident = cpool.tile([128, 128], F32)
        maskT = cpool.tile([128, 128], F32)
        lmask = cpool.tile([128, 256], F32)
        Ml_sb = cpool.tile([128, L, 128], F32)
        bdWkT = cpool.tile([128, L, 128], F32)
        bdWvT = cpool.tile([128, L, 128], F32)
        ones_row = cpool.tile([1, 128], F32)
        w5b = cpool.tile([128, 5], F32)
        nc.sync.dma_start(ident[:], ident_d[:])
        nc.sync.dma_start(maskT[:], maskT_d[:])
        nc.sync.dma_start(lmask[:], lmask_d[:])
        nc.sync.dma_start(Ml_sb[:], Ml_d.ap().rearrange("l p c -> p l c"))
        nc.sync.dma_start(bdWkT[:], bdWkT_d.ap().rearrange("l p c -> p l c"))
        nc.sync.dma_start(bdWvT[:], bdWvT_d.ap().rearrange("l p c -> p l c"))
        nc.vector.memset(ones_row[:], 1.0)

        glob = persist.tile([128, N], F32)
        loc = persist.tile([128, N], F32)

        with ExitStack() as phAB:
            keep = phAB.enter_context(tc.tile_pool(name="keep", bufs=1))
            qT = keep.tile([128, N], F32)
            klT = keep.tile([128, N], F32)
            qpT = keep.tile([128, N], F32)
            kpT = keep.tile([128, N], F32)
            knat = keep.tile([128, N], F32)
            kpnat = keep.tile([128, N], F32)
            vaug = keep.tile([128, 2 * NCH, 65], F32)
            vlaug = keep.tile([128, 2 * NCH, 65], F32)
            vnat = keep.tile([128, N], F32)
            S_sb = keep.tile([128, 5, 65], F32)

            with ExitStack() as phA:
                trans = phA.enter_context(tc.tile_pool(name="trans", bufs=1))
                ps_tr = phA.enter_context(
                    tc.tile_pool(name="ps_tr", bufs=3, space="PSUM"))
                phX = phA.enter_context(ExitStack())
                xT_p = phX.enter_context(tc.tile_pool(name="xTp", bufs=1))
                xnat_p = phX.enter_context(tc.tile_pool(name="xnat", bufs=3))
                wp_p = phX.enter_context(tc.tile_pool(name="wproj", bufs=2))
                ps_mm = phX.enter_context(
                    tc.tile_pool(name="ps_mm", bufs=2, space="PSUM"))

                # softmax(haar_scale) -> w5b = [1, sw0..sw3] broadcast down
                hs = cpool.tile([1, L], F32)
                nc.sync.dma_start(hs[:], hs_d[:])
                e4 = cpool.tile([1, L], F32)
                s1 = cpool.tile([1, 1], F32)
                nc.scalar.activation(e4[:], hs[:], AF.Exp, accum_out=s1[:])
                r1 = cpool.tile([1, 1], F32)
                nc.vector.reciprocal(r1[:], s1[:])
                w5 = cpool.tile([1, 5], F32)
                nc.vector.memset(w5[:, 0:1], 1.0)
                nc.vector.tensor_scalar_mul(w5[:, 1:5], e4[:], r1[:])
                w5bp = ps_tr.tile([128, 5], F32, tag="ptr")
                nc.tensor.matmul(w5bp[:], ones_row[:], w5[:],
                                 start=True, stop=True)
                nc.any.tensor_copy(w5b[:], w5bp[:])

                # ----- x^T -----
                xT = xT_p.tile([128, 8, N], F32)
                for i in range(NCH):
                    xn = xnat_p.tile([128, DM], F32, tag="xnat")
                    nc.sync.dma_start(xn[:], x_d[CH * i:CH * (i + 1), :])
                    for k in range(8):
                        pt = ps_tr.tile([128, 128], F32, tag="ptr")
                        nc.tensor.transpose(
                            pt[:], xn[:, 128 * k:128 * (k + 1)], ident[:])
                        nc.any.tensor_copy(xT[:, k, CH * i:CH * (i + 1)],
                                           pt[:])

                # ----- projections -----
                kTt = trans.tile([128, N], F32)
                vTt = trans.tile([128, N], F32)
                vlTt = trans.tile([128, N], F32)
                for p, dst in (("q", qT), ("k", kTt), ("v", vTt),
                               ("kl", klT), ("vl", vlTt)):
                    wsb = wp_p.tile([128, 8, 128], F32, tag="w")
                    nc.sync.dma_start(
                        wsb[:], wT[p].ap().rearrange("(k p) m -> p k m", p=128))
                    for nb in range(4):
                        acc = ps_mm.tile([128, 512], F32, tag="pacc")
                        for k in range(8):
                            nc.tensor.matmul(
                                acc[:], wsb[:, k, :],
                                xT[:, k, 512 * nb:512 * (nb + 1)],
                                start=(k == 0), stop=(k == 7))
                        nc.any.tensor_copy(dst[:, 512 * nb:512 * (nb + 1)],
                                           acc[:])

                phX.close()
                tmp_p = phA.enter_context(tc.tile_pool(name="phitmp", bufs=2))

                # ----- phi(q), phi(k) -----
                def phi_big(dst, src):
                    tmp = tmp_p.tile([128, N], F32, tag="phitmp")
                    nc.vector.tensor_scalar_min(tmp[:], src[:], 0.0)
                    nc.scalar.activation(dst[:], tmp[:], AF.Exp)
                    nc.vector.scalar_tensor_tensor(
                        dst[:], src[:], 0.0, dst[:], op0=OP.max, op1=OP.add)

                phi_big(qpT, qT)
                phi_big(kpT, kTt)

                # ----- natural layouts via PE transpose -----
                nc.vector.memset(vaug[:, :, 64:65], 1.0)
                nc.vector.memset(vlaug[:, :, 64:65], 1.0)
                for i in range(NCH):
                    sl = slice(CH * i, CH * (i + 1))
                    for src, dst in ((kTt, knat), (kpT, kpnat)):
                        pt = ps_tr.tile([128, 128], F32, tag="ptr")
                        nc.tensor.transpose(pt[:], src[:, sl], ident[:])
                        nc.any.tensor_copy(dst[:, sl], pt[:])
                    for src, dst in ((vTt, vaug), (vlTt, vlaug)):
                        pt = ps_tr.tile([128, 128], F32, tag="ptr")
                        nc.tensor.transpose(pt[:], src[:, sl], ident[:])
                        for h in range(2):
                            nc.any.tensor_copy(dst[:, 2 * i + h, 0:64],
                                               pt[:, 64 * h:64 * h + 64])
                        if dst is vaug:
                            nc.any.tensor_copy(vnat[:, sl], pt[:])

            # ----- chunk-major recurrence + local attention -----
            bm_p = phAB.enter_context(tc.tile_pool(name="bm", bufs=3))
            lvl_p = phAB.enter_context(tc.tile_pool(name="lvl", bufs=6))
            atm_p = phAB.enter_context(tc.tile_pool(name="atm", bufs=3))
            tin_p = phAB.enter_context(tc.tile_pool(name="tiny", bufs=4))
            ps_A = phAB.enter_context(
                tc.tile_pool(name="ps_A", bufs=2, space="PSUM"))
            ps_O = phAB.enter_context(
                tc.tile_pool(name="ps_O", bufs=2, space="PSUM"))
            ps_Sd = phAB.enter_context(
                tc.tile_pool(name="ps_Sd", bufs=1, space="PSUM"))
            ps_Lo = phAB.enter_context(
                tc.tile_pool(name="ps_Lo", bufs=1, space="PSUM"))
            ps_h = phAB.enter_context(
                tc.tile_pool(name="ps_h", bufs=2, space="PSUM"))

            def phi_small(psrc, tag):
                tmp = tin_p.tile([128, 128], F32, tag="phs")
                nc.vector.tensor_scalar_min(tmp[:], psrc[:], 0.0)
                dst = lvl_p.tile([128, 128], F32, tag=tag)
                nc.scalar.activation(dst[:], tmp[:], AF.Exp)
                nc.vector.scalar_tensor_tensor(
                    dst[:], psrc[:], 0.0, dst[:], op0=OP.max, op1=OP.add)
                return dst

            for i in range(NCH):
                sl = slice(CH * i, CH * (i + 1))
                kplT, kplN, vlvA = [], [], []
                for lv in range(L):
                    bmk_ps = ps_h.tile([128, 128], F32, tag="psh")
                    nc.tensor.matmul(bmk_ps[:], knat[:, sl], Ml_sb[:, lv, :],
                                     start=True, stop=True)
                    bmk = bm_p.tile([128, 128], F32, tag="bmk")
                    nc.any.tensor_copy(bmk[:], bmk_ps[:])
                    bmv_ps = ps_h.tile([128, 128], F32, tag="psh")
                    nc.tensor.matmul(bmv_ps[:], vnat[:, sl],
                                     Ml_sb[:, lv, :], start=True, stop=True)
                    bmv = bm_p.tile([128, 128], F32, tag="bmv")
                    nc.any.tensor_copy(bmv[:], bmv_ps[:])

                    kt_ps = ps_h.tile([128, 128], F32, tag="psh")
                    nc.tensor.matmul(kt_ps[:], bdWkT[:, lv, :], bmk[:],
                                     start=True, stop=True)
                    kplT.append(phi_small(kt_ps, "kplT"))
                    kn_ps = ps_h.tile([128, 128], F32, tag="psh")
                    nc.tensor.matmul(kn_ps[:], bmk[:], bdWkT[:, lv, :],
                                     start=True, stop=True)
                    kplN.append(phi_small(kn_ps, "kplN"))
                    vn_ps = ps_h.tile([128, 128], F32, tag="psh")
                    nc.tensor.matmul(vn_ps[:], bmv[:], bdWvT[:, lv, :],
                                     start=True, stop=True)
                    va = lvl_p.tile([128, 2, 65], F32, tag="vlv")
                    nc.vector.memset(va[:, :, 64:65], 1.0)
                    for h in range(2):
                        nc.any.tensor_copy(va[:, h, 0:64],
                                           vn_ps[:, 64 * h:64 * h + 64])
                    vlvA.append(va)

                psSd = ps_Sd.tile([128, 5, 65], F32, tag="psSd")
                for h in range(2):
                    hp = slice(64 * h, 64 * h + 64)
                    psO = ps_O.tile([128, 5, 65], F32, tag="psO")
                    for lv in range(5):
                        if lv == 0:
                            kpT_l = kpT[hp, sl]
                            va_l = vaug[:, 2 * i + h, :]
                        else:
                            kpT_l = kplT[lv - 1][hp, :]
                            va_l = vlvA[lv - 1][:, h, :]
                        psA = ps_A.tile([128, 128], F32, tag="psA")
                        nc.tensor.matmul(psA[:], kpT_l, qpT[hp, sl],
                                         start=True, stop=True)
                        atm = atm_p.tile([128, 128], F32, tag="atm")
                        nc.vector.tensor_mul(atm[:], psA[:], maskT[:])
                        nc.tensor.matmul(psO[:, lv, :], atm[:], va_l,
                                         start=True, stop=(i == 0))
                        if i > 0:
                            nc.tensor.matmul(psO[:, lv, :], qpT[hp, sl],
                                             S_sb[hp, lv, :],
                                             start=False, stop=True)
                    dmax = tin_p.tile([128, 5], F32, tag="dmax")
                    nc.vector.tensor_scalar_max(dmax[:], psO[:, :, 64], EPS)
                    rec = tin_p.tile([128, 5], F32, tag="rec")
                    nc.vector.reciprocal(rec[:], dmax[:])
                    rw = tin_p.tile([128, 5], F32, tag="rw")
                    nc.vector.tensor_mul(rw[:], rec[:], w5b[:])
                    gsl = glob[:, CH * i + 64 * h:CH * i + 64 * h + 64]
                    nc.vector.tensor_scalar_mul(gsl, psO[:, 0, 0:64],
                                                rw[:, 0:1])
                    for lv in range(1, 5):
                        nc.vector.scalar_tensor_tensor(
                            gsl, psO[:, lv, 0:64], rw[:, lv:lv + 1], gsl,
                            op0=OP.mult, op1=OP.add)
                    for lv in range(5):
                        if lv == 0:
                            kn_l = kpnat[:, CH * i + 64 * h:CH * i + 64 * h + 64]
                            va_l = vaug[:, 2 * i + h, :]
                        else:
                            kn_l = kplN[lv - 1][:, hp]
                            va_l = vlvA[lv - 1][:, h, :]
                        nc.tensor.matmul(psSd[hp, lv, :], kn_l, va_l,
                                         start=True, stop=True)
                if i == 0:
                    nc.vector.tensor_copy(S_sb[:], psSd[:])
                else:
                    nc.vector.tensor_add(S_sb[:], S_sb[:], psSd[:])

                for h in range(2):
                    hp = slice(64 * h, 64 * h + 64)
                    psL = ps_A.tile([128, 256], F32, tag="psA")
                    if i > 0:
                        nc.tensor.matmul(psL[:, 0:128],
                                         klT[hp, CH * (i - 1):CH * i],
                                         qT[hp, sl], start=True, stop=True)
                    nc.tensor.matmul(psL[:, 128:256], klT[hp, sl], qT[hp, sl],
                                     start=True, stop=True)
                    P = atm_p.tile([128, 256], F32, tag="P")
                    if i > 0:
                        nc.scalar.activation(P[:], psL[:], AF.Exp, scale=0.125)
                        nc.vector.tensor_mul(P[:], P[:], lmask[:])
                    else:
                        nc.scalar.activation(P[:, 128:256], psL[:, 128:256],
                                             AF.Exp, scale=0.125)
                        nc.vector.tensor_mul(P[:, 128:256], P[:, 128:256],
                                             lmask[:, 128:256])
                    psLo = ps_Lo.tile([128, 65], F32, tag="psLo")
                    if i > 0:
                        nc.tensor.matmul(psLo[:], P[:, 0:128],
                                         vlaug[:, 2 * (i - 1) + h, :],
                                         start=True, stop=False)
                    nc.tensor.matmul(psLo[:], P[:, 128:256],
                                     vlaug[:, 2 * i + h, :],
                                     start=(i == 0), stop=True)
                    dm = tin_p.tile([128, 1], F32, tag="dm")
                    nc.vector.tensor_scalar_max(dm[:], psLo[:, 64:65], 1e-30)
                    rl = tin_p.tile([128, 1], F32, tag="rl")
                    nc.vector.reciprocal(rl[:], dm[:])
                    nc.scalar.mul(loc[:, CH * i + 64 * h:CH * i + 64 * h + 64],
                                  psLo[:, 0:64], rl[:])

            nc.vector.tensor_sub(loc[:], loc[:], glob[:])
            for c2 in range(2):
                nc.sync.dma_start(
                    a2a_in.ap()[:, 0, c2].rearrange("j p m -> p j m"),
                    loc[:].rearrange("p (j c m) -> p j c m",
                                     c=2, m=128)[:, :, c2, :])
                nc.sync.dma_start(
                    a2a_in.ap()[:, 1, c2].rearrange("j p m -> p j m"),
                    glob[:].rearrange("p (j c m) -> p j c m",
                                      c=2, m=128)[:, :, c2, :])

        nc.gpsimd.collective_compute(
            "AllToAll", OP.bypass,
            ins=[a2a_in.ap().opt()], outs=[a2a_out.ap().opt()],
            replica_groups=[list(range(NCORES))])

        # ---------- sequence-parallel tail ----------
        with ExitStack() as phC:
            tl = phC.enter_context(tc.tile_pool(name="tail", bufs=1))
            wst = phC.enter_context(tc.tile_pool(name="wstream", bufs=3))
            ps_tr2 = phC.enter_context(
                tc.tile_pool(name="ps_tr2", bufs=2, space="PSUM"))
            ps_g = phC.enter_context(
                tc.tile_pool(name="ps_g", bufs=1, space="PSUM"))

            diff_g = tl.tile([128, 2, DM], F32)
            glob_g = tl.tile([128, 2, DM], F32)
            for t2 in range(2):
                nc.sync.dma_start(
                    diff_g[:, t2, :].rearrange("p (s m) -> p s m", s=8),
                    a2a_out.ap()[:, 0, t2].rearrange("s p m -> p s m"))
                nc.sync.dma_start(
                    glob_g[:, t2, :].rearrange("p (s m) -> p s m", s=8),
                    a2a_out.ap()[:, 1, t2].rearrange("s p m -> p s m"))

            pid = nc.sync.partition_id()
            row0 = pid * NSL
            xsl = tl.tile([128, 2, DM], F32)
            nc.sync.dma_start(
                xsl[:], x_d[bass.ds(row0, NSL), :].rearrange(
                    "(a b) c -> b a c", b=128))

            xslT = tl.tile([128, 8, 256], F32)
            diffT = tl.tile([128, 8, 256], F32)
            for t2 in range(2):
                for k in range(8):
                    pt = ps_tr2.tile([128, 128], F32, tag="ptr2")
                    nc.tensor.transpose(
                        pt[:], xsl[:, t2, 128 * k:128 * (k + 1)], ident[:])
                    nc.any.tensor_copy(xslT[:, k, 128 * t2:128 * (t2 + 1)],
                                       pt[:])
                    pt2 = ps_tr2.tile([128, 128], F32, tag="ptr2")
                    nc.tensor.transpose(
                        pt2[:], diff_g[:, t2, 128 * k:128 * (k + 1)], ident[:])
                    nc.any.tensor_copy(diffT[:, k, 128 * t2:128 * (t2 + 1)],
                                       pt2[:])

            bg_sb = tl.tile([1, DM], F32)
            bo_sb = tl.tile([1, DM], F32)
            bgo_sb = tl.tile([1, 1], F32)
            wgo_sb = tl.tile([128, 8], F32)
            nc.sync.dma_start(bg_sb[:], bg_d[:])
            nc.sync.dma_start(bo_sb[:], bo_d[:])
            nc.sync.dma_start(bgo_sb[:], bgo_d[:])
            nc.sync.dma_start(
                wgo_sb[:], wgo_d.ap().rearrange("(g p) o -> p (g o)", p=128))

            gh = tl.tile([128, 2, DM], F32)
            psG = []
            for j in range(4):
                psG_t = ps_g.tile([128, 512], F32, tag=f"psG{j}")
                psG.append(psG_t)
            for kc in range(16):
                wg_t = wst.tile([128, DM], F32, tag="wg")
                nc.sync.dma_start(wg_t[:], wgT_d[128 * kc:128 * (kc + 1), :])
                for t2 in range(2):
                    lhs = (xslT[:, kc, 128 * t2:128 * (t2 + 1)] if kc < 8
                           else diffT[:, kc - 8, 128 * t2:128 * (t2 + 1)])
                    for g2 in range(2):
                        nc.tensor.matmul(
                            psG[2 * t2 + g2][:], lhs,
                            wg_t[:, 512 * g2:512 * (g2 + 1)],
                            start=(kc == 0), stop=False)
            for t2 in range(2):
                for g2 in range(2):
                    nc.tensor.matmul(
                        psG[2 * t2 + g2][:], ones_row[:],
                        bg_sb[:, 512 * g2:512 * (g2 + 1)],
                        start=False, stop=True)
                    nc.scalar.activation(
                        gh[:, t2, 512 * g2:512 * (g2 + 1)],
                        psG[2 * t2 + g2][:], AF.Silu)

            ghT = tl.tile([128, 8, 256], F32)
            for t2 in range(2):
                for k in range(8):
                    pt = ps_tr2.tile([128, 128], F32, tag="ptr2")
                    nc.tensor.transpose(
                        pt[:], gh[:, t2, 128 * k:128 * (k + 1)], ident[:])
                    nc.any.tensor_copy(ghT[:, k, 128 * t2:128 * (t2 + 1)],
                                       pt[:])

            psAl = ps_tr2.tile([128, 2], F32, tag="psAl")
            for t2 in range(2):
                for gc in range(8):
                    nc.tensor.matmul(psAl[:, t2:t2 + 1],
                                     ghT[:, gc, 128 * t2:128 * (t2 + 1)],
                                     wgo_sb[:, gc:gc + 1],
                                     start=(gc == 0), stop=False)
                nc.tensor.matmul(psAl[:, t2:t2 + 1], ones_row[:], bgo_sb[:],
                                 start=False, stop=True)
            alpha = tl.tile([128, 2], F32)
            nc.scalar.activation(alpha[:], psAl[:], AF.Sigmoid)

            mx = tl.tile([128, 2, DM], F32)
            for t2 in range(2):
                nc.vector.scalar_tensor_tensor(
                    mx[:, t2, :], diff_g[:, t2, :], alpha[:, t2:t2 + 1],
                    glob_g[:, t2, :], op0=OP.mult, op1=OP.add)
            mxT = tl.tile([128, 8, 256], F32)
            for t2 in range(2):
                for k in range(8):
                    pt = ps_tr2.tile([128, 128], F32, tag="ptr2")
                    nc.tensor.transpose(
                        pt[:], mx[:, t2, 128 * k:128 * (k + 1)], ident[:])
                    nc.any.tensor_copy(mxT[:, k, 128 * t2:128 * (t2 + 1)],
                                       pt[:])

            out_sb = tl.tile([128, 2, DM], F32)
            psF = []
            for j in range(4):
                psF_t = ps_g.tile([128, 512], F32, tag=f"psG{j}")
                psF.append(psF_t)
            for kc in range(8):
                wo_t = wst.tile([128, DM], F32, tag="wo")
                nc.sync.dma_start(wo_t[:], woT_d[128 * kc:128 * (kc + 1), :])
                for t2 in range(2):
                    for o2 in range(2):
                        nc.tensor.matmul(
                            psF[2 * t2 + o2][:],
                            mxT[:, kc, 128 * t2:128 * (t2 + 1)],
                            wo_t[:, 512 * o2:512 * (o2 + 1)],
                            start=(kc == 0), stop=False)
            for t2 in range(2):
                for o2 in range(2):
                    nc.tensor.matmul(
                        psF[2 * t2 + o2][:], ones_row[:],
                        bo_sb[:, 512 * o2:512 * (o2 + 1)],
                        start=False, stop=True)
                    nc.any.tensor_copy(out_sb[:, t2, 512 * o2:512 * (o2 + 1)],
                                       psF[2 * t2 + o2][:])

            nc.sync.dma_start(
                out_d.ap().rearrange("(a b) c -> b a c", b=128), out_sb[:])

    nc.compile()
    return nc


def _prep_in_maps(x, Wq, Wk, Wv, Wkl, Wvl, haar_Wk, haar_Wv, haar_scale,
                  Wg, bg, Wgo, bgo, Wo, bo):
    ident, maskT, lmask, Ml = _host_constants()
    x2 = np.ascontiguousarray(np.asarray(x, dtype=np.float32).reshape(N, DM))
    bdWkT = np.stack([_blockdiag2(np.asarray(haar_Wk[lv], dtype=np.float32).T)
                      for lv in range(L)])
    bdWvT = np.stack([_blockdiag2(np.asarray(haar_Wv[lv], dtype=np.float32).T)
                      for lv in range(L)])
    wgT = np.ascontiguousarray(np.asarray(Wg, dtype=np.float32).T)
    woT = np.ascontiguousarray(np.asarray(Wo, dtype=np.float32).T)
    wgo = np.ascontiguousarray(
        np.asarray(Wgo, dtype=np.float32).reshape(1, DM).T)
    shared = {
        "x": x2, "bdWkT": bdWkT, "bdWvT": bdWvT, "Ml": Ml,
        "maskT": maskT, "lmask": lmask, "ident": ident,
        "wgT": wgT, "woT": woT, "wgo": wgo,
        "bg": np.asarray(bg, dtype=np.float32).reshape(1, DM),
        "bo": np.asarray(bo, dtype=np.float32).reshape(1, DM),
        "bgo": np.asarray(bgo, dtype=np.float32).reshape(1, 1),
        "hscale": np.asarray(haar_scale, dtype=np.float32).reshape(1, L),
    }
    in_maps = []
    for c in range(NCORES):
        sc = slice(128 * c, 128 * (c + 1))
        m = dict(shared)
        for nm, W in (("wqT", Wq), ("wkT", Wk), ("wvT", Wv),
                      ("wklT", Wkl), ("wvlT", Wvl)):
            m[nm] = np.ascontiguousarray(
                np.asarray(W, dtype=np.float32)[sc, :].T)
        in_maps.append(m)
    return in_maps


def kernel_run(inputs, trace=False):
    if "nc" not in _CACHE:
        _CACHE["nc"] = _build_nc()
    nc = _CACHE["nc"]
    in_maps = _prep_in_maps(**inputs)
    res = run_bass_kernel_spmd(nc, in_maps, list(range(NCORES)), trace=trace)
    out = np.concatenate([res.results[c]["out"] for c in range(NCORES)],
                         axis=0)
    return out.reshape(1, N, DM).astype(np.float32), res


def kernel(**inputs):
    out, _ = kernel_run(inputs, trace=False)
    return out



# revision 2
# speedup vs baseline: 2.4317x; 2.4317x over previous
"""HALE attention (local windowed SDPA + chunked causal linear attention with
multiscale Haar context + adaptive gate) on 8 Trainium2 NeuronCores.

Sharding (B=1, so no batch DP):
  - 16 heads -> 2 heads per core (tensor-parallel over heads), packed into the
    128-partition dim for the q/k/v/local projections, the chunked
    linear-attention recurrence, and the 4 Haar-level recurrences.
  - Tail (gate + out_proj) is sequence-parallel: two half AllToAlls
    redistribute the per-head outputs (diff=local-glob, glob) from head-sharded
    to row-sharded; core c handles rows [128c,128c+128) and [1024+128c, ...).
    The first A2A (rows 0..1023) overlaps with the second half of the
    recurrence. Output rows re-interleaved on the host.

Performance notes:
  - All matmul operands are bf16 (PE: 1 cycle/row vs 4 for fp32); PSUM
    accumulation stays fp32, normalizers/reciprocals stay fp32 on DVE.
  - x^T is precomputed on the host (kills 128 PE transposes + the natural-x
    load); softmax(haar_scale) is precomputed on the host.
  - Tail weights (Wg^T, Wo^T) are preloaded into SBUF at kernel start so the
    tail never waits on DMA.
"""

import numpy as np
import ml_dtypes
from contextlib import ExitStack

import concourse.bass as bass
import concourse.bacc as bacc
import concourse.tile as tile
import concourse.mybir as mybir
from concourse.bass_utils import run_bass_kernel_spmd

F32 = mybir.dt.float32
BF16 = mybir.dt.bfloat16
AF = mybir.ActivationFunctionType
OP = mybir.AluOpType
BF = ml_dtypes.bfloat16

NCORES = 8
N = 2048
DM = 1024
H = 16
DH = 64
L = 4
CH = 128
NCH = N // CH
WIN = 64
NSL = N // NCORES
EPS = 1e-6

_CACHE = {}


def _host_constants():
    ident = np.eye(128, dtype=np.float32)
    ck = np.arange(CH)[:, None]
    cq = np.arange(CH)[None, :]
    maskT = (ck <= cq).astype(np.float32)
    prev = (ck >= cq + WIN + 1).astype(np.float32)
    cur = ((ck <= cq) & (ck >= cq - (WIN - 1))).astype(np.float32)
    lmask = np.concatenate([prev, cur], axis=1)
    Ml = np.zeros((L, CH, CH), dtype=np.float32)
    for lv in range(L):
        b = 2 ** (lv + 1)
        m = np.arange(CH)[:, None]
        n = np.arange(CH)[None, :]
        Ml[lv] = np.where(((m // b) == (n // b)) & (m <= n),
                          1.0 / (n % b + 1.0), 0.0)
    return ident, maskT, lmask, Ml


def _blockdiag2(a):
    z = np.zeros((128, 128), dtype=np.float32)
    z[:64, :64] = a
    z[64:, 64:] = a
    return z


def _build_nc():
    nc = bacc.Bacc("TRN2", target_bir_lowering=False, debug=False,
                   num_devices=NCORES)

    xT_d = nc.dram_tensor("xT", [DM, N], BF16, kind="ExternalInput")
    wT = {p: nc.dram_tensor(f"w{p}T", [DM, 128], BF16, kind="ExternalInput")
          for p in ("q", "k", "v", "kl", "vl")}
    bdWkT_d = nc.dram_tensor("bdWkT", [L, 128, 128], BF16, kind="ExternalInput")
    bdWvT_d = nc.dram_tensor("bdWvT", [L, 128, 128], BF16, kind="ExternalInput")
    Ml_d = nc.dram_tensor("Ml", [128, L * 128], BF16, kind="ExternalInput")
    maskT_d = nc.dram_tensor("maskT", [128, 128], BF16, kind="ExternalInput")
    lmask_d = nc.dram_tensor("lmask", [128, 256], BF16, kind="ExternalInput")
    ident_d = nc.dram_tensor("ident", [128, 128], BF16, kind="ExternalInput")
    wgT_d = nc.dram_tensor("wgT", [2 * DM, DM], BF16, kind="ExternalInput")
    woT_d = nc.dram_tensor("woT", [DM, DM], BF16, kind="ExternalInput")
    wgo_d = nc.dram_tensor("wgo", [DM, 1], BF16, kind="ExternalInput")
    bg_d = nc.dram_tensor("bg", [1, DM], BF16, kind="ExternalInput")
    bo_d = nc.dram_tensor("bo", [1, DM], BF16, kind="ExternalInput")
    bgo_d = nc.dram_tensor("bgo", [1, 1], BF16, kind="ExternalInput")
    w5b_d = nc.dram_tensor("w5b", [128, 5], F32, kind="ExternalInput")
    out_d = nc.dram_tensor("out", [NSL, DM], F32, kind="ExternalOutput")

    # per half: [dest, tensor(diff,glob), 128 dims, 128 rows]
    a2a_in = [nc.dram_tensor(f"a2a_in{h}", [NCORES, 2, 128, 128], BF16)
              for h in range(2)]
    a2a_out = [nc.dram_tensor(f"a2a_out{h}", [NCORES, 2, 128, 128], BF16)
               for h in range(2)]

    with tile.TileContext(nc) as tc, ExitStack() as root:
        cpool = root.enter_context(tc.tile_pool(name="consts", bufs=1))
        persist = root.enter_context(tc.tile_pool(name="persist", bufs=1))

        ident = cpool.tile([128, 128], BF16)
        maskT = cpool.tile([128, 128], BF16)
        lmask = cpool.tile([128, 256], BF16)
        Ml_sb = cpool.tile([128, L, 128], BF16)
        bdWkT = cpool.tile([128, L, 128], BF16)
        bdWvT = cpool.tile([128, L, 128], BF16)
        ones_row = cpool.tile([1, 128], BF16)
        w5b = cpool.tile([128, 5], F32)
        nc.sync.dma_start(ident[:], ident_d[:])
        nc.sync.dma_start(maskT[:], maskT_d[:])
        nc.sync.dma_start(lmask[:], lmask_d[:])
        nc.sync.dma_start(Ml_sb[:], Ml_d.ap().rearrange("p (l c) -> p l c", c=128))
        nc.sync.dma_start(bdWkT[:], bdWkT_d.ap().rearrange("l p c -> p l c"))
        nc.sync.dma_start(bdWvT[:], bdWvT_d.ap().rearrange("l p c -> p l c"))
        nc.sync.dma_start(w5b[:], w5b_d[:])
        nc.vector.memset(ones_row[:], 1.0)

        # tail weights preloaded up front (no deps -> DMA runs during phase A)
        wg_sb = persist.tile([128, 16, DM], BF16)
        wo_sb = persist.tile([128, 8, DM], BF16)
        nc.sync.dma_start(
            wg_sb[:], wgT_d.ap().rearrange("(k p) m -> p k m", p=128))
        nc.sync.dma_start(
            wo_sb[:], woT_d.ap().rearrange("(k p) m -> p k m", p=128))
        bg_sb = persist.tile([1, DM], BF16)
        bo_sb = persist.tile([1, DM], BF16)
        bgo_sb = persist.tile([1, 1], BF16)
        wgo_sb = persist.tile([128, 8], BF16)
        nc.sync.dma_start(bg_sb[:], bg_d[:])
        nc.sync.dma_start(bo_sb[:], bo_d[:])
        nc.sync.dma_start(bgo_sb[:], bgo_d[:])
        nc.sync.dma_start(
            wgo_sb[:], wgo_d.ap().rearrange("(g p) o -> p (g o)", p=128))

        # x^T rows for this core's tail slice (two row blocks), via DMA
        pid = nc.sync.partition_id()
        xslT = persist.tile([128, 8, 256], BF16)
        for t2 in range(2):
            nc.sync.dma_start(
                xslT[:, :, 128 * t2:128 * (t2 + 1)],
                xT_d.ap().rearrange("(k p) n -> p k n", p=128)[
                    :, :, bass.ds(t2 * 1024 + pid * 128, 128)])

        glob = persist.tile([128, N], BF16)
        loc = persist.tile([128, N], BF16)

        with ExitStack() as phAB:
            keep = phAB.enter_context(tc.tile_pool(name="keep", bufs=1))
            qT = keep.tile([128, N], BF16)
            klT = keep.tile([128, N], BF16)
            qpT = keep.tile([128, N], BF16)
            kpT = keep.tile([128, N], BF16)
            knat = keep.tile([128, N], BF16)
            kpnat = keep.tile([128, N], BF16)
            vaug = keep.tile([128, 2 * NCH, 65], BF16)
            vlaug = keep.tile([128, 2 * NCH, 65], BF16)
            vnat = keep.tile([128, N], BF16)
            S_sb = keep.tile([128, 5, 65], F32)
            S_bf = keep.tile([128, 5, 65], BF16)

            with ExitStack() as phA:
                trans = phA.enter_context(tc.tile_pool(name="trans", bufs=1))
                ps_tr = phA.enter_context(
                    tc.tile_pool(name="ps_tr", bufs=3, space="PSUM"))
                phX = phA.enter_context(ExitStack())
                xT_p = phX.enter_context(tc.tile_pool(name="xTp", bufs=1))
                wp_p = phX.enter_context(tc.tile_pool(name="wproj", bufs=2))
                ps_mm = phX.enter_context(
                    tc.tile_pool(name="ps_mm", bufs=2, space="PSUM"))

                # ----- x^T from host -----
                xT = xT_p.tile([128, 8, N], BF16)
                nc.sync.dma_start(
                    xT[:], xT_d.ap().rearrange("(k p) n -> p k n", p=128))

                # ----- projections -----
                kTt = trans.tile([128, N], BF16)
                vTt = trans.tile([128, N], BF16)
                vlTt = trans.tile([128, N], BF16)
                for p, dst in (("q", qT), ("k", kTt), ("v", vTt),
                               ("kl", klT), ("vl", vlTt)):
                    wsb = wp_p.tile([128, 8, 128], BF16, tag="w")
                    nc.sync.dma_start(
                        wsb[:], wT[p].ap().rearrange("(k p) m -> p k m", p=128))
                    for nb in range(4):
                        acc = ps_mm.tile([128, 512], F32, tag="pacc")
                        for k in range(8):
                            nc.tensor.matmul(
                                acc[:], wsb[:, k, :],
                                xT[:, k, 512 * nb:512 * (nb + 1)],
                                start=(k == 0), stop=(k == 7))
                        nc.any.tensor_copy(dst[:, 512 * nb:512 * (nb + 1)],
                                           acc[:])

                phX.close()
                tmp_p = phA.enter_context(tc.tile_pool(name="phitmp", bufs=2))

                # ----- phi(q), phi(k) -----
                def phi_big(dst, src):
                    tmp = tmp_p.tile([128, N], BF16, tag="phitmp")
                    nc.vector.tensor_scalar_min(tmp[:], src[:], 0.0)
                    nc.scalar.activation(dst[:], tmp[:], AF.Exp)
                    nc.vector.scalar_tensor_tensor(
                        dst[:], src[:], 0.0, dst[:], op0=OP.max, op1=OP.add)

                phi_big(qpT, qT)
                phi_big(kpT, kTt)

                # ----- natural layouts via PE transpose -----
                nc.vector.memset(vaug[:, :, 64:65], 1.0)
                nc.vector.memset(vlaug[:, :, 64:65], 1.0)
                for i in range(NCH):
                    sl = slice(CH * i, CH * (i + 1))
                    for src, dst in ((kTt, knat), (kpT, kpnat)):
                        pt = ps_tr.tile([128, 128], BF16, tag="ptr")
                        nc.tensor.transpose(pt[:], src[:, sl], ident[:])
                        nc.any.tensor_copy(dst[:, sl], pt[:])
                    for src, dst in ((vTt, vaug), (vlTt, vlaug)):
                        pt = ps_tr.tile([128, 128], BF16, tag="ptr")
                        nc.tensor.transpose(pt[:], src[:, sl], ident[:])
                        for h in range(2):
                            nc.any.tensor_copy(dst[:, 2 * i + h, 0:64],
                                               pt[:, 64 * h:64 * h + 64])
                        if dst is vaug:
                            nc.any.tensor_copy(vnat[:, sl], pt[:])

            # ----- chunk-major recurrence + local attention -----
            bm_p = phAB.enter_context(tc.tile_pool(name="bm", bufs=3))
            lvl_p = phAB.enter_context(tc.tile_pool(name="lvl", bufs=6))
            atm_p = phAB.enter_context(tc.tile_pool(name="atm", bufs=3))
            tin_p = phAB.enter_context(tc.tile_pool(name="tiny", bufs=4))
            ps_A = phAB.enter_context(
                tc.tile_pool(name="ps_A", bufs=2, space="PSUM"))
            ps_O = phAB.enter_context(
                tc.tile_pool(name="ps_O", bufs=2, space="PSUM"))
            ps_Sd = phAB.enter_context(
                tc.tile_pool(name="ps_Sd", bufs=1, space="PSUM"))
            ps_Lo = phAB.enter_context(
                tc.tile_pool(name="ps_Lo", bufs=1, space="PSUM"))
            ps_h = phAB.enter_context(
                tc.tile_pool(name="ps_h", bufs=2, space="PSUM"))

            def phi_small(psrc, tag):
                tmp = tin_p.tile([128, 128], BF16, tag="phs")
                nc.vector.tensor_scalar_min(tmp[:], psrc[:], 0.0)
                dst = lvl_p.tile([128, 128], BF16, tag=tag)
                nc.scalar.activation(dst[:], tmp[:], AF.Exp)
                nc.vector.scalar_tensor_tensor(
                    dst[:], psrc[:], 0.0, dst[:], op0=OP.max, op1=OP.add)
                return dst

            def send_half(hh):
                # diff = loc - glob for rows [hh*1024, hh*1024+1024)
                hsl = slice(1024 * hh, 1024 * (hh + 1))
                nc.vector.tensor_sub(loc[:, hsl], loc[:, hsl], glob[:, hsl])
                nc.sync.dma_start(
                    a2a_in[hh].ap()[:, 0].rearrange("j p m -> p j m"),
                    loc[:, hsl].rearrange("p (j m) -> p j m", m=128))
                nc.sync.dma_start(
                    a2a_in[hh].ap()[:, 1].rearrange("j p m -> p j m"),
                    glob[:, hsl].rearrange("p (j m) -> p j m", m=128))
                nc.gpsimd.collective_compute(
                    "AllToAll", OP.bypass,
                    ins=[a2a_in[hh].ap().opt()], outs=[a2a_out[hh].ap().opt()],
                    replica_groups=[list(range(NCORES))])

            for i in range(NCH):
                sl = slice(CH * i, CH * (i + 1))
                kplT, kplN, vlvA = [], [], []
                for lv in range(L):
                    bmk_ps = ps_h.tile([128, 128], F32, tag="psh")
                    nc.tensor.matmul(bmk_ps[:], knat[:, sl], Ml_sb[:, lv, :],
                                     start=True, stop=True)
                    bmk = bm_p.tile([128, 128], BF16, tag="bmk")
                    nc.any.tensor_copy(bmk[:], bmk_ps[:])
                    bmv_ps = ps_h.tile([128, 128], F32, tag="psh")
                    nc.tensor.matmul(bmv_ps[:], vnat[:, sl],
                                     Ml_sb[:, lv, :], start=True, stop=True)
                    bmv = bm_p.tile([128, 128], BF16, tag="bmv")
                    nc.any.tensor_copy(bmv[:], bmv_ps[:])

                    kt_ps = ps_h.tile([128, 128], F32, tag="psh")
                    nc.tensor.matmul(kt_ps[:], bdWkT[:, lv, :], bmk[:],
                                     start=True, stop=True)
                    kplT.append(phi_small(kt_ps, "kplT"))
                    kn_ps = ps_h.tile([128, 128], F32, tag="psh")
                    nc.tensor.matmul(kn_ps[:], bmk[:], bdWkT[:, lv, :],
                                     start=True, stop=True)
                    kplN.append(phi_small(kn_ps, "kplN"))
                    vn_ps = ps_h.tile([128, 128], F32, tag="psh")
                    nc.tensor.matmul(vn_ps[:], bmv[:], bdWvT[:, lv, :],
                                     start=True, stop=True)
                    va = lvl_p.tile([128, 2, 65], BF16, tag="vlv")
                    nc.vector.memset(va[:, :, 64:65], 1.0)
                    for h in range(2):
                        nc.any.tensor_copy(va[:, h, 0:64],
                                           vn_ps[:, 64 * h:64 * h + 64])
                    vlvA.append(va)

                psSd = ps_Sd.tile([128, 5, 65], F32, tag="psSd")
                for h in range(2):
                    hp = slice(64 * h, 64 * h + 64)
                    psO = ps_O.tile([128, 5, 65], F32, tag="psO")
                    for lv in range(5):
                        if lv == 0:
                            kpT_l = kpT[hp, sl]
                            va_l = vaug[:, 2 * i + h, :]
                        else:
                            kpT_l = kplT[lv - 1][hp, :]
                            va_l = vlvA[lv - 1][:, h, :]
                        psA = ps_A.tile([128, 128], F32, tag="psA")
                        nc.tensor.matmul(psA[:], kpT_l, qpT[hp, sl],
                                         start=True, stop=True)
                        atm = atm_p.tile([128, 128], BF16, tag="atm")
                        nc.vector.tensor_mul(atm[:], psA[:], maskT[:])
                        nc.tensor.matmul(psO[:, lv, :], atm[:], va_l,
                                         start=True, stop=(i == 0))
                        if i > 0:
                            nc.tensor.matmul(psO[:, lv, :], qpT[hp, sl],
                                             S_bf[hp, lv, :],
                                             start=False, stop=True)
                    dmax = tin_p.tile([128, 5], F32, tag="dmax")
                    nc.vector.tensor_scalar_max(dmax[:], psO[:, :, 64], EPS)
                    rec = tin_p.tile([128, 5], F32, tag="rec")
                    nc.vector.reciprocal(rec[:], dmax[:])
                    rw = tin_p.tile([128, 5], F32, tag="rw")
                    nc.vector.tensor_mul(rw[:], rec[:], w5b[:])
                    gsl = glob[:, CH * i + 64 * h:CH * i + 64 * h + 64]
                    nc.vector.tensor_scalar_mul(gsl, psO[:, 0, 0:64],
                                                rw[:, 0:1])
                    for lv in range(1, 5):
                        nc.vector.scalar_tensor_tensor(
                            gsl, psO[:, lv, 0:64], rw[:, lv:lv + 1], gsl,
                            op0=OP.mult, op1=OP.add)
                    for lv in range(5):
                        if lv == 0:
                            kn_l = kpnat[:, CH * i + 64 * h:CH * i + 64 * h + 64]
                            va_l = vaug[:, 2 * i + h, :]
                        else:
                            kn_l = kplN[lv - 1][:, hp]
                            va_l = vlvA[lv - 1][:, h, :]
                        nc.tensor.matmul(psSd[hp, lv, :], kn_l, va_l,
                                         start=True, stop=True)
                if i == 0:
                    nc.vector.tensor_copy(S_sb[:], psSd[:])
                else:
                    nc.vector.tensor_add(S_sb[:], S_sb[:], psSd[:])
                nc.vector.tensor_copy(S_bf[:], S_sb[:])

                for h in range(2):
                    hp = slice(64 * h, 64 * h + 64)
                    psL = ps_A.tile([128, 256], F32, tag="psA")
                    if i > 0:
                        nc.tensor.matmul(psL[:, 0:128],
                                         klT[hp, CH * (i - 1):CH * i],
                                         qT[hp, sl], start=True, stop=True)
                    nc.tensor.matmul(psL[:, 128:256], klT[hp, sl], qT[hp, sl],
                                     start=True, stop=True)
                    P = atm_p.tile([128, 256], BF16, tag="P")
                    if i > 0:
                        nc.scalar.activation(P[:], psL[:], AF.Exp, scale=0.125)
                        nc.vector.tensor_mul(P[:], P[:], lmask[:])
                    else:
                        nc.scalar.activation(P[:, 128:256], psL[:, 128:256],
                                             AF.Exp, scale=0.125)
                        nc.vector.tensor_mul(P[:, 128:256], P[:, 128:256],
                                             lmask[:, 128:256])
                    psLo = ps_Lo.tile([128, 65], F32, tag="psLo")
                    if i > 0:
                        nc.tensor.matmul(psLo[:], P[:, 0:128],
                                         vlaug[:, 2 * (i - 1) + h, :],
                                         start=True, stop=False)
                    nc.tensor.matmul(psLo[:], P[:, 128:256],
                                     vlaug[:, 2 * i + h, :],
                                     start=(i == 0), stop=True)
                    dm = tin_p.tile([128, 1], F32, tag="dm")
                    nc.vector.tensor_scalar_max(dm[:], psLo[:, 64:65], 1e-30)
                    rl = tin_p.tile([128, 1], F32, tag="rl")
                    nc.vector.reciprocal(rl[:], dm[:])
                    nc.scalar.mul(loc[:, CH * i + 64 * h:CH * i + 64 * h + 64],
                                  psLo[:, 0:64], rl[:])

                if i == NCH // 2 - 1:
                    send_half(0)
            send_half(1)

        # ---------- row-parallel tail ----------
        with ExitStack() as phC:
            tl = phC.enter_context(tc.tile_pool(name="tail", bufs=1))
            ps_tr2 = phC.enter_context(
                tc.tile_pool(name="ps_tr2", bufs=2, space="PSUM"))
            ps_g = phC.enter_context(
                tc.tile_pool(name="ps_g", bufs=1, space="PSUM"))

            diff_g = tl.tile([128, 2, DM], BF16)
            glob_g = tl.tile([128, 2, DM], BF16)
            for t2 in range(2):
                nc.sync.dma_start(
                    diff_g[:, t2, :].rearrange("p (s m) -> p s m", s=8),
                    a2a_out[t2].ap()[:, 0].rearrange("s p m -> p s m"))
                nc.sync.dma_start(
                    glob_g[:, t2, :].rearrange("p (s m) -> p s m", s=8),
                    a2a_out[t2].ap()[:, 1].rearrange("s p m -> p s m"))

            diffT = tl.tile([128, 8, 256], BF16)
            for t2 in range(2):
                for k in range(8):
                    pt2 = ps_tr2.tile([128, 128], BF16, tag="ptr2")
                    nc.tensor.transpose(
                        pt2[:], diff_g[:, t2, 128 * k:128 * (k + 1)], ident[:])
                    nc.any.tensor_copy(diffT[:, k, 128 * t2:128 * (t2 + 1)],
                                       pt2[:])

            gh = tl.tile([128, 2, DM], BF16)
            psG = []
            for j in range(4):
                psG_t = ps_g.tile([128, 512], F32, tag=f"psG{j}")
                psG.append(psG_t)
            for kc in range(16):
                for t2 in range(2):
                    lhs = (xslT[:, kc, 128 * t2:128 * (t2 + 1)] if kc < 8
                           else diffT[:, kc - 8, 128 * t2:128 * (t2 + 1)])
                    for g2 in range(2):
                        nc.tensor.matmul(
                            psG[2 * t2 + g2][:], lhs,
                            wg_sb[:, kc, 512 * g2:512 * (g2 + 1)],
                            start=(kc == 0), stop=False)
            for t2 in range(2):
                for g2 in range(2):
                    nc.tensor.matmul(
                        psG[2 * t2 + g2][:], ones_row[:],
                        bg_sb[:, 512 * g2:512 * (g2 + 1)],
                        start=False, stop=True)
                    nc.scalar.activation(
                        gh[:, t2, 512 * g2:512 * (g2 + 1)],
                        psG[2 * t2 + g2][:], AF.Silu)

            ghT = tl.tile([128, 8, 256], BF16)
            for t2 in range(2):
                for k in range(8):
                    pt = ps_tr2.tile([128, 128], BF16, tag="ptr2")
                    nc.tensor.transpose(
                        pt[:], gh[:, t2, 128 * k:128 * (k + 1)], ident[:])
                    nc.any.tensor_copy(ghT[:, k, 128 * t2:128 * (t2 + 1)],
                                       pt[:])

            psAl = ps_tr2.tile([128, 2], F32, tag="psAl")
            for t2 in range(2):
                for gc in range(8):
                    nc.tensor.matmul(psAl[:, t2:t2 + 1],
                                     ghT[:, gc, 128 * t2:128 * (t2 + 1)],
                                     wgo_sb[:, gc:gc + 1],
                                     start=(gc == 0), stop=False)
                nc.tensor.matmul(psAl[:, t2:t2 + 1], ones_row[:], bgo_sb[:],
                                 start=False, stop=True)
            alpha = tl.tile([128, 2], F32)
            nc.scalar.activation(alpha[:], psAl[:], AF.Sigmoid)

            mx = tl.tile([128, 2, DM], BF16)
            for t2 in range(2):
                nc.vector.scalar_tensor_tensor(
                    mx[:, t2, :], diff_g[:, t2, :], alpha[:, t2:t2 + 1],
                    glob_g[:, t2, :], op0=OP.mult, op1=OP.add)
            mxT = tl.tile([128, 8, 256], BF16)
            for t2 in range(2):
                for k in range(8):
                    pt = ps_tr2.tile([128, 128], BF16, tag="ptr2")
                    nc.tensor.transpose(
                        pt[:], mx[:, t2, 128 * k:128 * (k + 1)], ident[:])
                    nc.any.tensor_copy(mxT[:, k, 128 * t2:128 * (t2 + 1)],
                                       pt[:])

            out_sb = tl.tile([128, 2, DM], F32)
            psF = []
            for j in range(4):
                psF_t = ps_g.tile([128, 512], F32, tag=f"psG{j}")
                psF.append(psF_t)
            for kc in range(8):
                for t2 in range(2):
                    for o2 in range(2):
                        nc.tensor.matmul(
                            psF[2 * t2 + o2][:],
                            mxT[:, kc, 128 * t2:128 * (t2 + 1)],
                            wo_sb[:, kc, 512 * o2:512 * (o2 + 1)],
                            start=(kc == 0), stop=False)
            for t2 in range(2):
                for o2 in range(2):
                    nc.tensor.matmul(
                        psF[2 * t2 + o2][:], ones_row[:],
                        bo_sb[:, 512 * o2:512 * (o2 + 1)],
                        start=False, stop=True)
                    nc.any.tensor_copy(out_sb[:, t2, 512 * o2:512 * (o2 + 1)],
                                       psF[2 * t2 + o2][:])

            nc.sync.dma_start(
                out_d.ap().rearrange("(a b) c -> b a c", b=128), out_sb[:])

    nc.compile()
    return nc


def _prep_in_maps(x, Wq, Wk, Wv, Wkl, Wvl, haar_Wk, haar_Wv, haar_scale,
                  Wg, bg, Wgo, bgo, Wo, bo):
    ident, maskT, lmask, Ml = _host_constants()
    x2 = np.asarray(x, dtype=np.float32).reshape(N, DM)
    xT = np.ascontiguousarray(x2.T).astype(BF)
    bdWkT = np.stack([_blockdiag2(np.asarray(haar_Wk[lv], dtype=np.float32).T)
                      for lv in range(L)]).astype(BF)
    bdWvT = np.stack([_blockdiag2(np.asarray(haar_Wv[lv], dtype=np.float32).T)
                      for lv in range(L)]).astype(BF)
    wgT = np.ascontiguousarray(np.asarray(Wg, dtype=np.float32).T).astype(BF)
    woT = np.ascontiguousarray(np.asarray(Wo, dtype=np.float32).T).astype(BF)
    wgo = np.ascontiguousarray(
        np.asarray(Wgo, dtype=np.float32).reshape(1, DM).T).astype(BF)
    hs = np.asarray(haar_scale, dtype=np.float64)
    sw = np.exp(hs - hs.max())
    sw = (sw / sw.sum()).astype(np.float32)
    w5 = np.concatenate([[1.0], sw]).astype(np.float32)
    w5b = np.broadcast_to(w5, (128, 5)).copy()
    Ml_all = np.concatenate([Ml[lv] for lv in range(L)], axis=1).astype(BF)
    shared = {
        "xT": xT, "bdWkT": bdWkT, "bdWvT": bdWvT, "Ml": Ml_all,
        "maskT": maskT.astype(BF), "lmask": lmask.astype(BF),
        "ident": ident.astype(BF),
        "wgT": wgT, "woT": woT, "wgo": wgo, "w5b": w5b,
        "bg": np.asarray(bg, dtype=np.float32).reshape(1, DM).astype(BF),
        "bo": np.asarray(bo, dtype=np.float32).reshape(1, DM).astype(BF),
        "bgo": np.asarray(bgo, dtype=np.float32).reshape(1, 1).astype(BF),
    }
    in_maps = []
    for c in range(NCORES):
        sc = slice(128 * c, 128 * (c + 1))
        m = dict(shared)
        for nm, W in (("wqT", Wq), ("wkT", Wk), ("wvT", Wv),
                      ("wklT", Wkl), ("wvlT", Wvl)):
            m[nm] = np.ascontiguousarray(
                np.asarray(W, dtype=np.float32)[sc, :].T).astype(BF)
        in_maps.append(m)
    return in_maps


def kernel_run(inputs, trace=False):
    if "nc" not in _CACHE:
        _CACHE["nc"] = _build_nc()
    nc = _CACHE["nc"]
    in_maps = _prep_in_maps(**inputs)
    res = run_bass_kernel_spmd(nc, in_maps, list(range(NCORES)), trace=trace)
    out = np.zeros((N, DM), dtype=np.float32)
    for c in range(NCORES):
        r = res.results[c]["out"]
        out[128 * c:128 * (c + 1)] = r[0:128]
        out[1024 + 128 * c:1024 + 128 * (c + 1)] = r[128:256]
    return out.reshape(1, N, DM), res


def kernel(**inputs):
    out, _ = kernel_run(inputs, trace=False)
    return out


# revision 36
# speedup vs baseline: 3.5093x; 1.4431x over previous
"""HALE attention (local windowed SDPA + chunked causal linear attention with
multiscale Haar context + adaptive gate) on 8 Trainium2 NeuronCores.

Sharding (B=1, so no batch DP):
  - 16 heads -> 2 heads per core (tensor-parallel over heads), packed into the
    128-partition dim for the q/k/v/local projections, the chunked
    linear-attention recurrence, and the 4 Haar-level recurrences.
  - Tail (gate + out_proj) is row-parallel: two half AllToAlls redistribute
    the per-head outputs (diff=local-glob, glob) from head-sharded to
    row-sharded; core c handles rows [128c,128c+128) and [1024+128c, ...).
    The first A2A (rows 0..1023) overlaps the second half of the recurrence;
    the tail's first half overlaps the second A2A. Rows re-interleaved on host.

Performance notes:
  - All matmul operands bf16 (PE 1 cycle/row vs 4 for fp32); PSUM stays fp32;
    normalizers/reciprocals fp32 on DVE.
  - x^T and softmax(haar_scale) precomputed on the host; x^T DMA split per
    128-dim block so the projections start as soon as the first block lands.
  - Phase B batches work to cut instruction count: block-means for all 4 Haar
    levels in one matmul (free=512), per-level phi on level-batched [128,512]
    tiles, per-head score matmuls merged via block-diagonal q (free=256),
    state updates head-packed (free=130), the 5 state contributions to the
    output in one matmul (free=325), local-attention scores/exp/mask merged
    across heads ([128,512] each).
  - Tail weights preloaded into SBUF during phases A/B.
"""

import numpy as np
import ml_dtypes
from contextlib import ExitStack

import concourse.bass as bass
import concourse.bacc as bacc
import concourse.tile as tile
import concourse.mybir as mybir
from concourse.bass_utils import run_bass_kernel_spmd

F32 = mybir.dt.float32
BF16 = mybir.dt.bfloat16
AF = mybir.ActivationFunctionType
OP = mybir.AluOpType
BF = ml_dtypes.bfloat16

NCORES = 8
N = 2048
DM = 1024
H = 16
DH = 64
L = 4
CH = 128
NCH = N // CH
WIN = 64
NSL = N // NCORES
EPS = 1e-6

_CACHE = {}


def _host_constants():
    ident = np.eye(128, dtype=np.float32)
    ck = np.arange(CH)[:, None]
    cq = np.arange(CH)[None, :]
    maskT = (ck <= cq).astype(np.float32)
    prev = (ck >= cq + WIN + 1).astype(np.float32)
    cur = ((ck <= cq) & (ck >= cq - (WIN - 1))).astype(np.float32)
    # psA layout: [c', (h, c)] x2 levels -> tile maskT 4x
    maskA = np.tile(maskT, (1, 4))
    # psL2 layout: [c', (block in {prev,cur}, h, c)]
    lmask2 = np.concatenate([prev, prev, cur, cur], axis=1)
    Ml = np.zeros((L, CH, CH), dtype=np.float32)
    for lv in range(L):
        b = 2 ** (lv + 1)
        m = np.arange(CH)[:, None]
        n = np.arange(CH)[None, :]
        Ml[lv] = np.where(((m // b) == (n // b)) & (m <= n),
                          1.0 / (n % b + 1.0), 0.0)
    Ml_all = np.concatenate([Ml[lv] for lv in range(L)], axis=1)
    return ident, maskA, lmask2, Ml_all


def _blockdiag2(a):
    z = np.zeros((128, 128), dtype=np.float32)
    z[:64, :64] = a
    z[64:, 64:] = a
    return z


def _build_nc():
    nc = bacc.Bacc("TRN2", target_bir_lowering=False, debug=False,
                   num_devices=NCORES)

    xT_d = nc.dram_tensor("xT", [DM, N], BF16, kind="ExternalInput")
    wT = {p: nc.dram_tensor(f"w{p}T", [DM, 128], BF16, kind="ExternalInput")
          for p in ("q", "k", "v", "kl", "vl")}
    bdWkT_d = nc.dram_tensor("bdWkT", [L, 128, 128], BF16, kind="ExternalInput")
    bdWvT_d = nc.dram_tensor("bdWvT", [L, 128, 128], BF16, kind="ExternalInput")
    Ml_d = nc.dram_tensor("Ml", [128, L * 128], BF16, kind="ExternalInput")
    maskA_d = nc.dram_tensor("maskA", [128, 512], BF16, kind="ExternalInput")
    lmask2_d = nc.dram_tensor("lmask2", [128, 512], BF16, kind="ExternalInput")
    ident_d = nc.dram_tensor("ident", [128, 128], BF16, kind="ExternalInput")
    wgT_d = nc.dram_tensor("wgT", [2 * DM, DM], BF16, kind="ExternalInput")
    woT_d = nc.dram_tensor("woT", [DM, DM], BF16, kind="ExternalInput")
    wgo_d = nc.dram_tensor("wgo", [128, DM], BF16, kind="ExternalInput")
    bg_d = nc.dram_tensor("bg", [1, DM], BF16, kind="ExternalInput")
    bo_d = nc.dram_tensor("bo", [1, DM], BF16, kind="ExternalInput")
    bgo_d = nc.dram_tensor("bgo", [128, 1], F32, kind="ExternalInput")
    w5b_d = nc.dram_tensor("w5b", [128, 10], F32, kind="ExternalInput")
    out_d = nc.dram_tensor("out", [NSL, DM], F32, kind="ExternalOutput")

    # per half: [dest, tensor(diff,glob), 128 dims, 128 rows]
    a2a_in = [nc.dram_tensor(f"a2a_in{h}", [NCORES, 2, 128, 128], BF16)
              for h in range(2)]
    a2a_out = [nc.dram_tensor(f"a2a_out{h}", [NCORES, 2, 128, 128], BF16)
               for h in range(2)]

    with tile.TileContext(nc) as tc, ExitStack() as root:
        cpool = root.enter_context(tc.tile_pool(name="consts", bufs=1))
        persist = root.enter_context(tc.tile_pool(name="persist", bufs=1))

        # ---- x^T first (everything waits on it), split per 128-dim block ----
        xT = persist.tile([128, 8, N], BF16)
        xT_r = xT_d.ap().rearrange("(k p) n -> p k n", p=128)
        for k in range(8):
            nc.sync.dma_start(xT[:, k, :], xT_r[:, k, :])

        ident = cpool.tile([128, 128], BF16)
        maskA = cpool.tile([128, 512], BF16)
        lmask2 = cpool.tile([128, 512], BF16)
        Ml_sb = cpool.tile([128, L * 128], BF16)
        bdWkT = cpool.tile([128, L, 128], BF16)
        bdWvT = cpool.tile([128, L, 128], BF16)
        ones_row = cpool.tile([1, 128], BF16)
        w5b = cpool.tile([128, 10], F32)
        nc.sync.dma_start(ident[:], ident_d[:])
        nc.sync.dma_start(maskA[:], maskA_d[:])
        nc.sync.dma_start(lmask2[:], lmask2_d[:])
        nc.sync.dma_start(Ml_sb[:], Ml_d[:])
        nc.sync.dma_start(bdWkT[:], bdWkT_d.ap().rearrange("l p c -> p l c"))
        nc.sync.dma_start(bdWvT[:], bdWvT_d.ap().rearrange("l p c -> p l c"))
        nc.sync.dma_start(w5b[:], w5b_d[:])
        nc.vector.memset(ones_row[:], 1.0)

        pid = nc.sync.partition_id()
        glob = persist.tile([128, N], F32)
        loc = persist.tile([128, N], F32)

        # tail weights / tail x^T slice (prefetched; not needed until tail)
        wg_sb = persist.tile([128, 16, DM], BF16)
        wo_sb = persist.tile([128, 8, DM], BF16)
        bg_sb = persist.tile([1, DM], BF16)
        bo_sb = persist.tile([1, DM], BF16)
        bgo_sb = persist.tile([128, 1], F32)
        wgo_sb = persist.tile([128, DM], BF16)
        xslT = persist.tile([128, 8, 256], BF16)

        def prefetch_tail():
            nc.sync.dma_start(
                wg_sb[:], wgT_d.ap().rearrange("(k p) m -> p k m", p=128))
            nc.sync.dma_start(
                wo_sb[:], woT_d.ap().rearrange("(k p) m -> p k m", p=128))
            nc.sync.dma_start(bg_sb[:], bg_d[:])
            nc.sync.dma_start(bo_sb[:], bo_d[:])
            nc.sync.dma_start(bgo_sb[:], bgo_d[:])
            nc.sync.dma_start(wgo_sb[:], wgo_d[:])
            for t2 in range(2):
                nc.sync.dma_start(
                    xslT[:, :, 128 * t2:128 * (t2 + 1)],
                    xT_r[:, :, bass.ds(t2 * 1024 + pid * 128, 128)])

        with ExitStack() as phAB:
            keep = phAB.enter_context(tc.tile_pool(name="keep", bufs=1))
            qT = keep.tile([128, N], BF16)
            klT = keep.tile([128, N], BF16)
            qpT = keep.tile([128, N], BF16)
            kpT = keep.tile([128, N], BF16)
            knat = keep.tile([128, N], BF16)
            kpnat = keep.tile([128, N], BF16)
            vaug = keep.tile([128, 2 * NCH, 65], BF16)
            vlaug = keep.tile([128, 2 * NCH, 65], BF16)
            vnat = keep.tile([128, N], BF16)
            S_sb = keep.tile([128, 5, 130], F32)
            S_bf = keep.tile([128, 5, 130], BF16)

            with ExitStack() as phA:
                trans = phA.enter_context(tc.tile_pool(name="trans", bufs=1))
                ps_tr = phA.enter_context(
                    tc.tile_pool(name="ps_tr", bufs=3, space="PSUM"))
                phX = phA.enter_context(ExitStack())
                wp_p = phX.enter_context(tc.tile_pool(name="wproj", bufs=2))
                ps_mm = phX.enter_context(
                    tc.tile_pool(name="ps_mm", bufs=1, space="PSUM"))

                # ----- projections (k-outer: start as soon as xT block 0
                # lands; 4 live PSUM accumulators) -----
                kTt = trans.tile([128, N], BF16)
                vTt = trans.tile([128, N], BF16)
                vlTt = trans.tile([128, N], BF16)
                projs = (("q", qT), ("k", kTt), ("v", vTt),
                         ("kl", klT), ("vl", vlTt))
                wsb5 = {}
                for p, _ in projs:
                    wsb5[p] = wp_p.tile([128, 8, 128], BF16, tag=f"w{p}",
                                        name=f"wsb_{p}")
                    nc.sync.dma_start(
                        wsb5[p][:],
                        wT[p].ap().rearrange("(k p) m -> p k m", p=128))
                for p, dst in projs:
                    for nb in range(4):
                        acc = ps_mm.tile([128, 512], F32, tag="pacc", bufs=2)
                        for k in range(8):
                            nc.tensor.matmul(
                                acc[:], wsb5[p][:, k, :],
                                xT[:, k, 512 * nb:512 * (nb + 1)],
                                start=(k == 0), stop=(k == 7))
                        nc.vector.tensor_copy(dst[:, 512 * nb:512 * (nb + 1)],
                                              acc[:])

                phX.close()
                prefetch_tail()
                tmp_p = phA.enter_context(tc.tile_pool(name="phitmp", bufs=2))

                # ----- phi(q), phi(k) -----
                def phi_big(dst, src):
                    tmp = tmp_p.tile([128, N], BF16, tag="phitmp")
                    nc.vector.tensor_scalar_min(tmp[:], src[:], 0.0)
                    nc.scalar.activation(dst[:], tmp[:], AF.Exp)
                    nc.vector.scalar_tensor_tensor(
                        dst[:], src[:], 0.0, dst[:], op0=OP.max, op1=OP.add)

                phi_big(qpT, qT)
                phi_big(kpT, kTt)

                # ----- natural layouts via PE transpose -----
                nc.vector.memset(vaug[:, :, 64:65], 1.0)
                nc.vector.memset(vlaug[:, :, 64:65], 1.0)
                for i in range(NCH):
                    sl = slice(CH * i, CH * (i + 1))
                    pt = ps_tr.tile([128, 128], BF16, tag="ptr")
                    nc.tensor.transpose(pt[:], kTt[:, sl], ident[:])
                    nc.any.tensor_copy(knat[:, sl], pt[:])
                    ptv = ps_tr.tile([128, 128], BF16, tag="ptr")
                    nc.tensor.transpose(ptv[:], vTt[:, sl], ident[:])
                    nc.any.tensor_copy(vnat[:, sl], ptv[:])
                    nc.any.tensor_copy(
                        vaug[:, 2 * i:2 * i + 2, 0:64],
                        ptv[:].rearrange("p (h e) -> p h e", h=2))
                    ptl = ps_tr.tile([128, 128], BF16, tag="ptr")
                    nc.tensor.transpose(ptl[:], vlTt[:, sl], ident[:])
                    nc.any.tensor_copy(
                        vlaug[:, 2 * i:2 * i + 2, 0:64],
                        ptl[:].rearrange("p (h e) -> p h e", h=2))

                phi_big(kpnat, knat)

            # ----- chunk-major recurrence + local attention -----
            sb_p = phAB.enter_context(tc.tile_pool(name="sbB", bufs=3))
            atm_p = phAB.enter_context(tc.tile_pool(name="atm", bufs=3))
            tin_p = phAB.enter_context(tc.tile_pool(name="tiny", bufs=4))
            ps_b = phAB.enter_context(
                tc.tile_pool(name="ps_b", bufs=5, space="PSUM"))
            ps_O = phAB.enter_context(
                tc.tile_pool(name="ps_O", bufs=2, space="PSUM"))
            ps_Lo = phAB.enter_context(
                tc.tile_pool(name="ps_Lo", bufs=1, space="PSUM"))

            # double-buffered blockdiag q tiles and level-va tiles
            bdq2 = [keep.tile([128, 256], BF16, name=f"bdq{j}")
                    for j in range(2)]
            bdqr2 = [keep.tile([128, 256], BF16, name=f"bdqr{j}")
                     for j in range(2)]
            vlv2 = [keep.tile([128, L, 2, 65], BF16, name=f"vlv{j}")
                    for j in range(2)]
            for j in range(2):
                nc.vector.memset(bdq2[j][:], 0.0)
                nc.vector.memset(bdqr2[j][:], 0.0)
                nc.vector.memset(vlv2[j][:, :, :, 64:65], 1.0)

            def phi_batch(dst, psrc):
                # phi(x) = exp(min(x,0)) + max(x,0); min via ACT Relu(-x)
                tmp = sb_p.tile([128, 512], BF16, tag="phtmp")
                nc.scalar.activation(tmp[:], psrc[:], AF.Relu, scale=-1.0)
                nc.scalar.activation(dst[:], tmp[:], AF.Exp, scale=-1.0)
                nc.vector.scalar_tensor_tensor(
                    dst[:], psrc[:], 0.0, dst[:], op0=OP.max, op1=OP.add)

            def send_half(hh):
                # diff = loc - glob for rows [hh*1024, hh*1024+1024)
                hsl = slice(1024 * hh, 1024 * (hh + 1))
                nc.vector.tensor_sub(loc[:, hsl], loc[:, hsl], glob[:, hsl])
                nc.gpsimd.dma_start(
                    a2a_in[hh].ap()[:, 0].rearrange("j p m -> p j m"),
                    loc[:, hsl].rearrange("p (j m) -> p j m", m=128))
                nc.gpsimd.dma_start(
                    a2a_in[hh].ap()[:, 1].rearrange("j p m -> p j m"),
                    glob[:, hsl].rearrange("p (j m) -> p j m", m=128))
                nc.gpsimd.collective_compute(
                    "AllToAll", OP.bypass,
                    ins=[a2a_in[hh].ap().opt()], outs=[a2a_out[hh].ap().opt()],
                    replica_groups=[list(range(NCORES))])

            for i in range(NCH):
                sl = slice(CH * i, CH * (i + 1))
                bdq, bdqr, vlv = bdq2[i % 2], bdqr2[i % 2], vlv2[i % 2]
                # blockdiag q / phi(q) for head-merged score matmuls
                nc.any.tensor_copy(bdq[0:64, 0:128], qpT[0:64, sl])
                nc.any.tensor_copy(bdq[64:128, 128:256], qpT[64:128, sl])
                nc.any.tensor_copy(bdqr[0:64, 0:128], qT[0:64, sl])
                nc.any.tensor_copy(bdqr[64:128, 128:256], qT[64:128, sl])

                # ----- Haar level prep (all 4 levels batched) -----
                bmk_ps = ps_b.tile([128, 512], F32, tag="b512")
                nc.tensor.matmul(bmk_ps[:], knat[:, sl], Ml_sb[:],
                                 start=True, stop=True)
                bmk = sb_p.tile([128, 512], BF16, tag="bmk")
                nc.any.tensor_copy(bmk[:], bmk_ps[:])
                bmv_ps = ps_b.tile([128, 512], F32, tag="b512")
                nc.tensor.matmul(bmv_ps[:], vnat[:, sl], Ml_sb[:],
                                 start=True, stop=True)
                bmv = sb_p.tile([128, 512], BF16, tag="bmv")
                nc.any.tensor_copy(bmv[:], bmv_ps[:])

                kT4_ps = ps_b.tile([128, 512], F32, tag="b512")
                kN4_ps = ps_b.tile([128, 512], F32, tag="b512")
                vN4_ps = ps_b.tile([128, 512], F32, tag="b512")
                for lv in range(L):
                    lsl = slice(128 * lv, 128 * (lv + 1))
                    nc.tensor.matmul(kT4_ps[:, lsl], bdWkT[:, lv, :],
                                     bmk[:, lsl], start=True, stop=True)
                    nc.tensor.matmul(kN4_ps[:, lsl], bmk[:, lsl],
                                     bdWkT[:, lv, :], start=True, stop=True)
                    nc.tensor.matmul(vN4_ps[:, lsl], bmv[:, lsl],
                                     bdWvT[:, lv, :], start=True, stop=True)
                kplT = sb_p.tile([128, 512], BF16, tag="kplT")
                phi_batch(kplT, kT4_ps)
                kplN = sb_p.tile([128, 512], BF16, tag="kplN")
                phi_batch(kplN, kN4_ps)
                nc.vector.tensor_copy(
                    vlv[:, :, :, 0:64],
                    vN4_ps[:].rearrange("p (l h e) -> p l h e", l=L, h=2))

                # ----- scores for 5 levels, heads merged via blockdiag q -----
                psA = []
                for j in range(3):
                    psA.append(ps_b.tile([128, 512], F32, tag="b512",
                                         name=f"psA{j}"))
                for lv in range(5):
                    stat = kpT[:, sl] if lv == 0 else kplT[:, 128 * (lv - 1):
                                                          128 * lv]
                    nc.tensor.matmul(
                        psA[lv // 2][:, 256 * (lv % 2):256 * (lv % 2) + 256],
                        stat, bdq[:], start=True, stop=True)
                atm = []
                for j in range(3):
                    w = 512 if j < 2 else 256
                    a = atm_p.tile([128, w], BF16, tag=f"atm{j}")
                    nc.vector.tensor_mul(a[:], psA[j][:, 0:w], maskA[:, 0:w])
                    atm.append(a)

                # ----- per-head outputs + normalizers -----
                for h in range(2):
                    hp = slice(64 * h, 64 * h + 64)
                    psO = ps_O.tile([128, 5, 65], F32, tag="psO",
                                    name=f"psO{h}")
                    if i > 0:
                        # state contribution opens the accumulation group for
                        # the whole tile (one matmul, free=325)
                        nc.tensor.matmul(psO[:], qpT[hp, sl],
                                         S_bf[hp, :, 65 * h:65 * h + 65],
                                         start=True, stop=False)
                    for lv in range(5):
                        a = atm[lv // 2]
                        stat = a[:, 256 * (lv % 2) + 128 * h:
                                 256 * (lv % 2) + 128 * h + 128]
                        va_l = (vaug[:, 2 * i + h, :] if lv == 0
                                else vlv[:, lv - 1, h, :])
                        nc.tensor.matmul(psO[:, lv, :], stat, va_l,
                                         start=(i == 0), stop=(i == 0)
                                         or (lv == 4))
                    dmax = tin_p.tile([128, 5], F32, tag="dmax")
                    nc.vector.tensor_scalar_max(dmax[:], psO[:, :, 64], EPS)
                    rec = tin_p.tile([128, 5], F32, tag="rec")
                    nc.vector.reciprocal(rec[:], dmax[:])
                    rw = tin_p.tile([128, 5], F32, tag="rw")
                    nc.vector.tensor_mul(rw[:], rec[:],
                                         w5b[:, 5 * h:5 * h + 5])
                    tmpn = tin_p.tile([128, 5, 64], F32, tag="tmpn")
                    nc.vector.tensor_mul(
                        tmpn[:], psO[:, :, 0:64],
                        rw[:].broadcast_to([128, 5, 64]))
                    nc.vector.tensor_reduce(
                        glob[:, CH * i + 64 * h:CH * i + 64 * h + 64],
                        tmpn[:].rearrange("p l e -> p e l"),
                        axis=mybir.AxisListType.X, op=OP.add)

                # ----- state update, head-packed (free=130 per level) -----
                psSd_a = ps_b.tile([128, 512], F32, tag="b512")
                psSd_b = ps_b.tile([128, 512], F32, tag="b512")
                for lv in range(5):
                    kn_l = (kpnat[:, sl] if lv == 0
                            else kplN[:, 128 * (lv - 1):128 * lv])
                    va2 = (vaug[:, 2 * i:2 * i + 2, :] if lv == 0
                           else vlv[:, lv - 1, :, :])
                    dst = (psSd_a[:, 130 * lv:130 * lv + 130] if lv < 3
                           else psSd_b[:, 130 * (lv - 3):130 * (lv - 3) + 130])
                    nc.tensor.matmul(dst, kn_l, va2, start=True, stop=True)
                srcA = psSd_a[:, 0:390].rearrange("p (l e) -> p l e", e=130)
                srcB = psSd_b[:, 0:260].rearrange("p (l e) -> p l e", e=130)
                if i == 0:
                    nc.vector.tensor_copy(S_sb[:, 0:3, :], srcA)
                    nc.vector.tensor_copy(S_sb[:, 3:5, :], srcB)
                else:
                    nc.vector.tensor_add(S_sb[:, 0:3, :], S_sb[:, 0:3, :],
                                         srcA)
                    nc.vector.tensor_add(S_sb[:, 3:5, :], S_sb[:, 3:5, :],
                                         srcB)
                nc.vector.tensor_copy(S_bf[:], S_sb[:])

                # ----- local attention (heads merged) -----
                psL2 = ps_b.tile([128, 512], F32, tag="b512")
                if i > 0:
                    nc.tensor.matmul(psL2[:, 0:256],
                                     klT[:, CH * (i - 1):CH * i],
                                     bdqr[:], start=True, stop=True)
                nc.tensor.matmul(psL2[:, 256:512], klT[:, sl], bdqr[:],
                                 start=True, stop=True)
                P = atm_p.tile([128, 512], BF16, tag="P")
                if i > 0:
                    nc.scalar.activation(P[:], psL2[:], AF.Exp, scale=0.125)
                    nc.vector.tensor_mul(P[:], P[:], lmask2[:])
                else:
                    nc.scalar.activation(P[:, 256:512], psL2[:, 256:512],
                                         AF.Exp, scale=0.125)
                    nc.vector.tensor_mul(P[:, 256:512], P[:, 256:512],
                                         lmask2[:, 256:512])
                psLo = ps_Lo.tile([128, 2, 65], F32, tag="psLo")
                for h in range(2):
                    if i > 0:
                        nc.tensor.matmul(psLo[:, h, :],
                                         P[:, 128 * h:128 * h + 128],
                                         vlaug[:, 2 * (i - 1) + h, :],
                                         start=True, stop=False)
                    nc.tensor.matmul(psLo[:, h, :],
                                     P[:, 256 + 128 * h:256 + 128 * h + 128],
                                     vlaug[:, 2 * i + h, :],
                                     start=(i == 0), stop=True)
                dm = tin_p.tile([128, 2], F32, tag="dm")
                nc.vector.tensor_scalar_max(dm[:], psLo[:, :, 64], 1e-30)
                rl = tin_p.tile([128, 2], F32, tag="rl")
                nc.vector.reciprocal(rl[:], dm[:])
                for h in range(2):
                    nc.scalar.mul(loc[:, CH * i + 64 * h:CH * i + 64 * h + 64],
                                  psLo[:, h, 0:64], rl[:, h:h + 1])

                if i == NCH // 2 - 1:
                    send_half(0)
            send_half(1)

        # ---------- row-parallel tail: two independent halves ----------
        with ExitStack() as phC:
            tl = phC.enter_context(tc.tile_pool(name="tail", bufs=1))
            ps_tr2 = phC.enter_context(
                tc.tile_pool(name="ps_tr2", bufs=2, space="PSUM"))
            ps_g = phC.enter_context(
                tc.tile_pool(name="ps_g", bufs=1, space="PSUM"))

            for t2 in range(2):
                diff_g = tl.tile([128, DM], BF16, tag="diff_g", bufs=2)
                glob_g = tl.tile([128, DM], BF16, tag="glob_g", bufs=2)
                nc.sync.dma_start(
                    diff_g[:].rearrange("p (s m) -> p s m", s=8),
                    a2a_out[t2].ap()[:, 0].rearrange("s p m -> p s m"))
                nc.sync.dma_start(
                    glob_g[:].rearrange("p (s m) -> p s m", s=8),
                    a2a_out[t2].ap()[:, 1].rearrange("s p m -> p s m"))

                diffT = tl.tile([128, 8, 128], BF16, tag="diffT", bufs=2)
                for k in range(8):
                    pt2 = ps_tr2.tile([128, 128], BF16, tag="ptr2")
                    nc.tensor.transpose(
                        pt2[:], diff_g[:, 128 * k:128 * (k + 1)], ident[:])
                    nc.any.tensor_copy(diffT[:, k, :], pt2[:])

                gh = tl.tile([128, DM], BF16, tag="gh", bufs=2)
                psG = [ps_g.tile([128, 512], F32, tag=f"psG{j}",
                                 name=f"psG{j}")
                       for j in range(2)]
                for kc in range(16):
                    lhs = (xslT[:, kc, 128 * t2:128 * (t2 + 1)] if kc < 8
                           else diffT[:, kc - 8, :])
                    for g2 in range(2):
                        nc.tensor.matmul(
                            psG[g2][:], lhs,
                            wg_sb[:, kc, 512 * g2:512 * (g2 + 1)],
                            start=(kc == 0), stop=False)
                for g2 in range(2):
                    nc.tensor.matmul(
                        psG[g2][:], ones_row[:],
                        bg_sb[:, 512 * g2:512 * (g2 + 1)],
                        start=False, stop=True)
                    nc.scalar.activation(
                        gh[:, 512 * g2:512 * (g2 + 1)],
                        psG[g2][:], AF.Silu)

                # alpha logit = rowwise <gh, wgo> via broadcast-mul + reduce
                tal = tl.tile([128, DM], F32, tag="tal", bufs=2)
                nc.vector.tensor_mul(tal[:], gh[:], wgo_sb[:])
                logit = tl.tile([128, 1], F32, tag="logit", bufs=2)
                nc.vector.tensor_reduce(logit[:], tal[:],
                                        axis=mybir.AxisListType.X, op=OP.add)
                alpha = tl.tile([128, 1], F32, tag="alpha", bufs=2)
                nc.scalar.activation(alpha[:], logit[:], AF.Sigmoid,
                                     bias=bgo_sb[:])

                mx = tl.tile([128, DM], BF16, tag="mx", bufs=2)
                nc.vector.scalar_tensor_tensor(
                    mx[:], diff_g[:], alpha[:], glob_g[:],
                    op0=OP.mult, op1=OP.add)
                mxT = tl.tile([128, 8, 128], BF16, tag="mxT", bufs=2)
                for k in range(8):
                    pt = ps_tr2.tile([128, 128], BF16, tag="ptr2")
                    nc.tensor.transpose(
                        pt[:], mx[:, 128 * k:128 * (k + 1)], ident[:])
                    nc.any.tensor_copy(mxT[:, k, :], pt[:])

                out_sb = tl.tile([128, DM], F32, tag="out_sb", bufs=2)
                psF = [ps_g.tile([128, 512], F32, tag=f"psF{j}",
                                 name=f"psF{j}")
                       for j in range(2)]
                for kc in range(8):
                    for o2 in range(2):
                        nc.tensor.matmul(
                            psF[o2][:], mxT[:, kc, :],
                            wo_sb[:, kc, 512 * o2:512 * (o2 + 1)],
                            start=(kc == 0), stop=False)
                for o2 in range(2):
                    nc.tensor.matmul(
                        psF[o2][:], ones_row[:],
                        bo_sb[:, 512 * o2:512 * (o2 + 1)],
                        start=False, stop=True)
                    nc.any.tensor_copy(out_sb[:, 512 * o2:512 * (o2 + 1)],
                                       psF[o2][:])

                nc.sync.dma_start(
                    out_d.ap()[128 * t2:128 * (t2 + 1), :], out_sb[:])

    nc.compile()
    return nc


def _prep_in_maps(x, Wq, Wk, Wv, Wkl, Wvl, haar_Wk, haar_Wv, haar_scale,
                  Wg, bg, Wgo, bgo, Wo, bo):
    ident, maskA, lmask2, Ml_all = _host_constants()
    x2 = np.asarray(x, dtype=np.float32).reshape(N, DM)
    xT = np.ascontiguousarray(x2.T).astype(BF)
    bdWkT = np.stack([_blockdiag2(np.asarray(haar_Wk[lv], dtype=np.float32).T)
                      for lv in range(L)]).astype(BF)
    bdWvT = np.stack([_blockdiag2(np.asarray(haar_Wv[lv], dtype=np.float32).T)
                      for lv in range(L)]).astype(BF)
    wgT = np.ascontiguousarray(np.asarray(Wg, dtype=np.float32).T).astype(BF)
    woT = np.ascontiguousarray(np.asarray(Wo, dtype=np.float32).T).astype(BF)
    wgo = np.broadcast_to(np.asarray(Wgo, dtype=np.float32).reshape(1, DM),
                      (128, DM)).astype(BF)
    hs = np.asarray(haar_scale, dtype=np.float64)
    sw = np.exp(hs - hs.max())
    sw = (sw / sw.sum()).astype(np.float32)
    w5 = np.concatenate([[1.0], sw, [1.0], sw]).astype(np.float32)
    w5b = np.broadcast_to(w5, (128, 10)).copy()
    shared = {
        "xT": xT, "bdWkT": bdWkT, "bdWvT": bdWvT, "Ml": Ml_all.astype(BF),
        "maskA": maskA.astype(BF), "lmask2": lmask2.astype(BF),
        "ident": ident.astype(BF),
        "wgT": wgT, "woT": woT, "wgo": wgo, "w5b": w5b,
        "bg": np.asarray(bg, dtype=np.float32).reshape(1, DM).astype(BF),
        "bo": np.asarray(bo, dtype=np.float32).reshape(1, DM).astype(BF),
        "bgo": np.broadcast_to(
            np.asarray(bgo, dtype=np.float32).reshape(1, 1),
            (128, 1)).copy(),
    }
    in_maps = []
    for c in range(NCORES):
        sc = slice(128 * c, 128 * (c + 1))
        m = dict(shared)
        for nm, W in (("wqT", Wq), ("wkT", Wk), ("wvT", Wv),
                      ("wklT", Wkl), ("wvlT", Wvl)):
            m[nm] = np.ascontiguousarray(
                np.asarray(W, dtype=np.float32)[sc, :].T).astype(BF)
        in_maps.append(m)
    return in_maps


def kernel_run(inputs, trace=False):
    if "nc" not in _CACHE:
        _CACHE["nc"] = _build_nc()
    nc = _CACHE["nc"]
    in_maps = _prep_in_maps(**inputs)
    res = run_bass_kernel_spmd(nc, in_maps, list(range(NCORES)), trace=trace)
    out = np.zeros((N, DM), dtype=np.float32)
    for c in range(NCORES):
        r = res.results[c]["out"]
        out[128 * c:128 * (c + 1)] = r[0:128]
        out[1024 + 128 * c:1024 + 128 * (c + 1)] = r[128:256]
    return out.reshape(1, N, DM), res


def kernel(**inputs):
    out, _ = kernel_run(inputs, trace=False)
    return out
